# revision 1
# baseline (speedup 1.0000x reference)
"""Trainium2 Bass kernel for nn_CLLayer (SimCLR-style contrastive loss).

Math (reference, tau=0.5):
    h1 = elu(z1 @ W1.T + b1) @ W2.T + b2 ; h2 likewise
    n1, n2 = row-normalized h1, h2
    l1_i = log(sum_j exp(2*n1_i.n1_j) + sum_j exp(2*n1_i.n2_j) - e^2) - 2*n1_i.n2_i
    l2_i = log(sum_j exp(2*n2_i.n2_j) + sum_j exp(2*n2_j.n1_i... ) - e^2) - 2*...
    out = 0.5*(l1+l2)

Sharding: row-parallel over N=8192 (1024 rows/core, 8 cores).
Each core: projects its row block (bf16 matmuls), normalizes, AllGathers
normalized embeddings (bf16), computes its row-strip of the three distinct
similarity products (S12, S22, S11), exp+row-sums on the fly, column-sums of
exp(2*S12) via a ReduceScatter (between2 = between.T so l2's "between" row
sums are column sums of S12's exp).  Only 3 of 4 N^2*D products are needed.

Host-side prep: transposes z blocks / weights to K-major (PE wants K on
partitions), casts matmul operands to bf16, and folds the ELU "-1" into an
adjusted fc2 bias (b2' = b2 - fc2_w.sum(1)) so ELU is computed as
relu(x) + exp(min(x,0)) without the subtract (device ELU' = elu + 1).
"""

import math
import os
from functools import lru_cache

import ml_dtypes
import numpy as np

import concourse.bacc as bacc
import concourse.bass as bass
import concourse.mybir as mybir
import concourse.tile as tile
from concourse.bass_utils import run_bass_kernel_spmd

N, D = 8192, 1024
NCORES = 8
BLK = N // NCORES  # 1024
P = 128
KO = D // P  # 8 k-tiles
NT = BLK // P  # 8 i-tiles per core
JC = N // 512  # 16 j-chunks of 512
E2 = float(np.exp(2.0))  # exp(1/tau), tau=0.5
BF = mybir.dt.bfloat16
F32 = mybir.dt.float32
AF = mybir.ActivationFunctionType
ALU = mybir.AluOpType


def _build():
    nc = bacc.Bacc("TRN2", target_bir_lowering=False, debug=False, num_devices=NCORES)

    z1t = nc.dram_tensor("z1t", [D, BLK], BF, kind="ExternalInput")
    z2t = nc.dram_tensor("z2t", [D, BLK], BF, kind="ExternalInput")
    w1t = nc.dram_tensor("w1t", [D, D], BF, kind="ExternalInput")
    w2t = nc.dram_tensor("w2t", [D, D], BF, kind="ExternalInput")
    b1 = nc.dram_tensor("b1", [D], F32, kind="ExternalInput")
    b2p = nc.dram_tensor("b2p", [D], F32, kind="ExternalInput")
    out = nc.dram_tensor("out", [BLK], F32, kind="ExternalOutput")

    kp = lambda ap: ap.rearrange("(ko ki) x -> ki ko x", ki=P)  # K-major -> [128, KO, x]
    pt = lambda ap: ap.rearrange("(t p) -> p t", p=P)  # [1024] -> [128, 8]
    JP = JC // 2  # 8 j-chunk-pairs of 1024

    with tile.TileContext(nc) as tc:
        with (
            tc.tile_pool(name="consts", bufs=1) as consts,
            tc.tile_pool(name="mats", bufs=1) as mats,
            tc.tile_pool(name="strip", bufs=1) as strip,
            tc.tile_pool(name="scratch", bufs=2) as scratch,
            tc.tile_pool(name="rhs", bufs=3) as rhsp,
            tc.tile_pool(name="expp", bufs=2) as expp,
            tc.tile_pool(name="small", bufs=1) as small,
            tc.tile_pool(name="psA", bufs=3, space="PSUM") as psA,
            tc.tile_pool(name="psB", bufs=2, space="PSUM") as psB,
            tc.tile_pool(name="dram", bufs=1, space="DRAM") as dram,
        ):
            # ---------------- constants ----------------
            w1_sb = consts.tile([P, KO, D], BF)
            w2_sb = consts.tile([P, KO, D], BF)
            nc.sync.dma_start(w1_sb[:], kp(w1t[:]))
            nc.sync.dma_start(w2_sb[:], kp(w2t[:]))
            b1_sb = consts.tile([P, KO], F32)
            b2_sb = consts.tile([P, KO], F32)
            nc.sync.dma_start(b1_sb[:], pt(b1[:]))
            nc.sync.dma_start(b2_sb[:], pt(b2p[:]))
            ones_bf = consts.tile([P, 1], BF)
            ones_f = consts.tile([P, 1], F32)
            nc.vector.memset(ones_bf[:], 1.0)
            nc.vector.memset(ones_f[:], 1.0)

            z_sb = mats.tile([P, KO, BLK], BF, tag="zt")
            n1_sb = mats.tile([P, KO, BLK], BF, tag="n1")
            n2_sb = mats.tile([P, KO, BLK], BF, tag="n2")

            ag1_in = dram.tile([D, BLK], BF)
            ag2_in = dram.tile([D, BLK], BF)
            ag1_out = dram.tile([NCORES, D, BLK], BF, addr_space="Shared")
            ag2_out = dram.tile([NCORES, D, BLK], BF, addr_space="Shared")
            rs_in = dram.tile([N], F32)
            rs_out = dram.tile([BLK], F32)
            rn_dram = dram.tile([2, BLK], BF)
            p_dram = dram.tile([BLK], F32)

            # ------------ projection + normalize (into n_sb), per tensor ------------
            def project(z_at, elu_sb, n_sb, rn_slot):
                # layer 1: a1T[o, i] = W1T.T @ zT (K=d);
                # elu+1 = relu(a+b1) + min(exp(a+b1), 1)
                for ot in range(KO):
                    ps = psA.tile([P, 1024], F32, tag="ps_big")
                    for ch in range(2):
                        sl = bass.ts(ch, 512)
                        for kt in range(KO):
                            nc.tensor.matmul(
                                ps[:, sl],
                                w1_sb[:, kt, bass.ts(ot, P)],
                                z_at(kt, ch),
                                start=(kt == 0),
                                stop=(kt == KO - 1),
                            )
                    bcol = b1_sb[:, ot : ot + 1]
                    e_t = scratch.tile([P, 1024], F32, tag="e_t")
                    r_t = scratch.tile([P, 1024], F32, tag="r_t")
                    nc.scalar.activation(e_t[:], ps[:], AF.Exp, bias=bcol)
                    nc.scalar.activation(r_t[:], ps[:], AF.Relu, bias=bcol)
                    nc.vector.tensor_scalar(e_t[:], e_t[:], 1.0, None, ALU.min)
                    nc.vector.tensor_tensor(elu_sb[:, ot, :], e_t[:], r_t[:], ALU.add)
                # layer 2 -> n_sb (holds hT until scaled in place)
                for ot in range(KO):
                    ps = psA.tile([P, 1024], F32, tag="ps_big")
                    for ch in range(2):
                        sl = bass.ts(ch, 512)
                        for kt in range(KO):
                            nc.tensor.matmul(
                                ps[:, sl],
                                w2_sb[:, kt, bass.ts(ot, P)],
                                elu_sb[:, kt, bass.ds(ch * 512, 512)],
                                start=(kt == 0),
                                stop=(kt == KO - 1),
                            )
                    nc.vector.tensor_scalar(
                        n_sb[:, ot, :], ps[:], b2_sb[:, ot : ot + 1], None, ALU.add
                    )
                # sumsq over d (partitions) via ones-matmul on Square(h)
                ssps = [psB.tile([1, 512], F32, name=f"ssps{_c}", tag="ps_small") for _c in range(2)]
                for kt in range(KO):
                    sq = scratch.tile([P, BLK], BF, tag="sq")
                    nc.scalar.activation(sq[:], n_sb[:, kt, :], AF.Square)
                    for ch in range(2):
                        nc.tensor.matmul(
                            ssps[ch][:],
                            ones_bf[:],
                            sq[:, bass.ts(ch, 512)],
                            start=(kt == 0),
                            stop=(kt == KO - 1),
                        )
                # rn = 1/||h|| per column, one Newton step on top of 1/sqrt
                rn_bf = small.tile([1, BLK], BF, tag="rn_bf")
                for ch in range(2):
                    sl = bass.ts(ch, 512)
                    ssq_c = small.tile([1, 512], F32, tag="ssq_c", name=f"ssq_c{ch}")
                    nrm_c = small.tile([1, 512], F32, tag="nrm_c", name=f"nrm_c{ch}")
                    y_c = small.tile([1, 512], F32, tag="y_c", name=f"y_c{ch}")
                    t1_c = small.tile([1, 512], F32, tag="t1_c", name=f"t1_c{ch}")
                    nc.vector.tensor_copy(ssq_c[:], ssps[ch][:])
                    nc.scalar.activation(nrm_c[:], ssps[ch][:], AF.Sqrt)
                    nc.vector.reciprocal(y_c[:], nrm_c[:])
                    nc.vector.tensor_tensor(t1_c[:], y_c[:], y_c[:], ALU.mult)
                    nc.vector.tensor_tensor(t1_c[:], t1_c[:], ssq_c[:], ALU.mult)
                    nc.vector.tensor_scalar(t1_c[:], t1_c[:], -0.5, 1.5, ALU.mult, ALU.add)
                    nc.vector.tensor_tensor(t1_c[:], y_c[:], t1_c[:], ALU.mult)
                    nc.vector.tensor_copy(rn_bf[:, sl], t1_c[:])
                nc.scalar.dma_start(rn_dram[rn_slot : rn_slot + 1, :], rn_bf[:])
                rn_bc = scratch.tile([P, BLK], BF, tag="rnbc", bufs=1)
                nc.scalar.dma_start(rn_bc[:], rn_dram[rn_slot : rn_slot + 1, :].to_broadcast((P, BLK)))
                for kt in range(KO):
                    nc.vector.tensor_tensor(n_sb[:, kt, :], n_sb[:, kt, :], rn_bc[:], ALU.mult)

            rg = [list(range(NCORES))]
            # z1 into its slot; z2 into the (idle until pass A) rhs-pool slots so
            # both projections can interleave on the PE.
            nc.sync.dma_start(z_sb[:], kp(z1t[:]))
            z2a = rhsp.tile([P, KO, 512], BF, tag="rhs", name="z2a")
            z2b = rhsp.tile([P, KO, 512], BF, tag="rhs", name="z2b")
            nc.sync.dma_start(z2a[:], kp(z2t[:, 0:512]))
            nc.sync.dma_start(z2b[:], kp(z2t[:, 512:1024]))
            elu1 = mats.tile([P, KO, BLK], BF, tag="elu")
            project(lambda kt, ch: z_sb[:, kt, bass.ds(ch * 512, 512)], elu1, n1_sb, 0)
            nc.scalar.dma_start(kp(ag1_in[:]), n1_sb[:])
            nc.gpsimd.collective_compute(
                "AllGather", ALU.bypass, replica_groups=rg,
                ins=[ag1_in[:].opt()], outs=[ag1_out[:].opt()],
            )
            # elu2 reuses the z1 slot (z1 dead after its layer 1)
            elu2 = mats.tile([P, KO, BLK], BF, tag="zt", name="elu2")
            project(lambda kt, ch: (z2a if ch == 0 else z2b)[:, kt, :], elu2, n2_sb, 1)
            nc.scalar.dma_start(kp(ag2_in[:]), n2_sb[:])
            nc.gpsimd.collective_compute(
                "AllGather", ALU.bypass, replica_groups=rg,
                ins=[ag2_in[:].opt()], outs=[ag2_out[:].opt()],
            )

            # ---------------- p_i = n1_i . n2_i (local diag of S12) ----------------
            pps = [psB.tile([1, 512], F32, name=f"pps{_c}", tag="ps_small") for _c in range(2)]
            for kt in range(KO):
                q = scratch.tile([P, BLK], BF, tag="sq")
                nc.vector.tensor_tensor(q[:], n1_sb[:, kt, :], n2_sb[:, kt, :], ALU.mult)
                for ch in range(2):
                    nc.tensor.matmul(
                        pps[ch][:],
                        ones_bf[:],
                        q[:, bass.ts(ch, 512)],
                        start=(kt == 0),
                        stop=(kt == KO - 1),
                    )
            for ch in range(2):
                p_c = small.tile([1, 512], F32, tag="ssq_c", name=f"p_c{ch}")
                nc.vector.tensor_copy(p_c[:], pps[ch][:])
                nc.gpsimd.dma_start(p_dram[ch * 512 : (ch + 1) * 512], p_c[:])

            # rowsum partials, one column per j-chunk-pair
            r11p = strip.tile([P, NT, JP], F32)
            r12p = strip.tile([P, NT, JP], F32)
            r22p = strip.tile([P, NT, JP], F32)
            cs = strip.tile([P, N], F32)  # exp(2*S12) partial column sums

            def rhs_pair(ag, jp):
                a = rhsp.tile([P, KO, 512], BF, tag="rhs", name=f"rhs_a{jp}")
                b = rhsp.tile([P, KO, 512], BF, tag="rhs", name=f"rhs_b{jp}")
                blk = kp(ag[jp])
                nc.sync.dma_start(a[:], blk[:, :, 0:512])
                nc.sync.dma_start(b[:], blk[:, :, 512:1024])
                return a, b

            def sim_iter(lhs, tt, rta, rtb, accum, s12_jp=None):
                ps = psA.tile([P, 1024], F32, tag="ps_big", name="ps_sim")
                for ch, rt in ((0, rta), (1, rtb)):
                    sl = bass.ts(ch, 512)
                    for kt in range(KO):
                        nc.tensor.matmul(
                            ps[:, sl],
                            lhs[:, kt, bass.ts(tt, P)],
                            rt[:, kt, :],
                            start=(kt == 0),
                            stop=(kt == KO - 1),
                        )
                ex = expp.tile([P, 1024], F32, tag="ex")
                nc.scalar.activation(ex[:], ps[:], AF.Exp, scale=2.0, accum_out=accum)
                if s12_jp is not None:
                    csl = cs[:, bass.ds(s12_jp * 1024, 1024)]
                    nc.vector.tensor_tensor(csl, csl, ex[:], ALU.add)

            # ---- pass A: S11 (lhs n1, rhs gathered n1) ----
            for jp in range(JP):
                rta, rtb = rhs_pair(ag1_out, jp)
                for tt in range(NT):
                    sim_iter(n1_sb, tt, rta, rtb, r11p[:, tt, jp : jp + 1])

            # ---- pass B1: S12 (lhs n1, rhs gathered n2) + incremental colsums ----
            nc.vector.memset(cs[:], 0.0)
            for jp in range(JP):
                rta, rtb = rhs_pair(ag2_out, jp)
                for tt in range(NT):
                    sim_iter(n1_sb, tt, rta, rtb, r12p[:, tt, jp : jp + 1], s12_jp=jp)
                # this 1024-wide slice of cs is complete -> reduce over partitions
                for h in range(2):
                    cp = psB.tile([1, 512], F32, tag="ps_small", name=f"cp{jp}_{h}")
                    nc.tensor.matmul(
                        cp[:], ones_f[:], cs[:, bass.ds(jp * 1024 + h * 512, 512)],
                        start=True, stop=True,
                    )
                    cst = scratch.tile([1, 512], F32, tag="cst", bufs=2, name=f"cst{jp}_{h}")
                    nc.vector.tensor_copy(cst[:], cp[:])
                    nc.gpsimd.dma_start(
                        rs_in[(jp * 2 + h) * 512 : (jp * 2 + h + 1) * 512], cst[:]
                    )
            nc.gpsimd.collective_compute(
                "ReduceScatter", ALU.add, replica_groups=rg,
                ins=[rs_in[:].opt()], outs=[rs_out[:].opt()],
            )

            # ---- pass B2: S22 (lhs n2, rhs gathered n2); RS overlaps this ----
            for jp in range(JP):
                rta, rtb = rhs_pair(ag2_out, jp)
                for tt in range(NT):
                    sim_iter(n2_sb, tt, rta, rtb, r22p[:, tt, jp : jp + 1])

            # ---------------- final loss ----------------
            r11 = small.tile([P, NT], F32, tag="r11")
            r12 = small.tile([P, NT], F32, tag="r12")
            r22 = small.tile([P, NT], F32, tag="r22")
            nc.vector.reduce_sum(r11[:], r11p[:], axis=mybir.AxisListType.X)
            nc.vector.reduce_sum(r12[:], r12p[:], axis=mybir.AxisListType.X)
            nc.vector.reduce_sum(r22[:], r22p[:], axis=mybir.AxisListType.X)
            c12 = small.tile([P, NT], F32, tag="c12")
            nc.sync.dma_start(c12[:], pt(rs_out[:]))
            p2 = small.tile([P, NT], F32, tag="p2")
            nc.sync.dma_start(p2[:], pt(p_dram[:]))

            d1 = small.tile([P, NT], F32, tag="d1")
            d2 = small.tile([P, NT], F32, tag="d2")
            nc.vector.tensor_tensor(d1[:], r11[:], r12[:], ALU.add)
            nc.vector.tensor_scalar(d1[:], d1[:], -E2, None, ALU.add)
            nc.vector.tensor_tensor(d2[:], r22[:], c12[:], ALU.add)
            nc.vector.tensor_scalar(d2[:], d2[:], -E2, None, ALU.add)
            l1 = small.tile([P, NT], F32, tag="l1")
            l2 = small.tile([P, NT], F32, tag="l2")
            nc.scalar.activation(l1[:], d1[:], AF.Ln)
            nc.scalar.activation(l2[:], d2[:], AF.Ln)
            loss = small.tile([P, NT], F32, tag="loss")
            nc.vector.tensor_tensor(loss[:], l1[:], l2[:], ALU.add)
            nc.vector.tensor_scalar(loss[:], loss[:], 0.5, None, ALU.mult)
            pm = small.tile([P, NT], F32, tag="pm")
            nc.vector.tensor_scalar(pm[:], p2[:], -2.0, None, ALU.mult)
            nc.vector.tensor_tensor(loss[:], loss[:], pm[:], ALU.add)
            nc.sync.dma_start(pt(out[:]), loss[:])

    nc.finalize()
    return nc


@lru_cache(maxsize=1)
def _built():
    return _build()


def _prep_inputs(z1, z2, fc1_w, fc1_b, fc2_w, fc2_b):
    bf = ml_dtypes.bfloat16
    w1t = np.ascontiguousarray(np.asarray(fc1_w, np.float32).T).astype(bf)
    w2t = np.ascontiguousarray(np.asarray(fc2_w, np.float32).T).astype(bf)
    b1 = np.asarray(fc1_b, np.float32)
    b2p = (np.asarray(fc2_b, np.float32) - np.asarray(fc2_w, np.float32).sum(axis=1)).astype(
        np.float32
    )
    in_maps = []
    for c in range(NCORES):
        sl = slice(c * BLK, (c + 1) * BLK)
        in_maps.append(
            {
                "z1t": np.ascontiguousarray(np.asarray(z1[sl], np.float32).T).astype(bf),
                "z2t": np.ascontiguousarray(np.asarray(z2[sl], np.float32).T).astype(bf),
                "w1t": w1t,
                "w2t": w2t,
                "b1": b1,
                "b2p": b2p,
            }
        )
    return in_maps


def _install_ntff_shim():
    """Register the axon NTFF profile hook (antenv.axon_hooks is absent in
    this image; rebuild it from trn_agent_boot's ctypes recipe)."""
    import sys
    import types

    if "antenv.axon_hooks" in sys.modules:
        return True
    try:
        import antenv
        from trn_agent_boot.trn_boot import _ntff_profile_via_ctypes

        hook = _ntff_profile_via_ctypes("/opt/axon/libaxon_pjrt.so")
        if hook is None:
            return False
        m = types.ModuleType("antenv.axon_hooks")
        m._hook = hook
        m.get_axon_ntff_profile_hook = lambda: m._hook
        m.set_axon_ntff_profile_hook = lambda h: setattr(m, "_hook", h)
        sys.modules["antenv.axon_hooks"] = m
        antenv.axon_hooks = m
        # artifact upload needs egress; neuter it for local profiling
        import concourse.bass_utils as _bu

        _bu.upload_artifacts = lambda tmpdir: f"file://{tmpdir}"
        return True
    except Exception as e:
        print(f"ntff shim unavailable: {e!r}")
        return False


def _run(in_maps, trace=False):
    nc = _built()
    if trace and not _install_ntff_shim():
        trace = False
    last = None
    for attempt in range(3):
        try:
            res = run_bass_kernel_spmd(nc, in_maps, list(range(NCORES)), trace=trace)
            if all(np.isfinite(res.results[c]["out"]).all() for c in range(NCORES)):
                return res
            print("nonfinite output, retrying")
        except Exception as e:  # device occasionally wedged from a prior process
            last = e
            if "UNRECOVERABLE" not in str(e) and "UNAVAILABLE" not in str(e):
                raise
            print(f"device error (attempt {attempt}): retrying")
    if last is not None:
        raise last
    return res


def kernel(z1, z2, fc1_w, fc1_b, fc2_w, fc2_b):
    in_maps = _prep_inputs(z1, z2, fc1_w, fc1_b, fc2_w, fc2_b)
    res = _run(in_maps, trace=os.environ.get("KERNEL_TRACE", "") == "1")
    if res.exec_time_ns is not None:
        print(f"HW exec time: {res.exec_time_ns} ns")
    out = np.concatenate([res.results[c]["out"] for c in range(NCORES)])
    return out.astype(np.float32)



# revision 5
# speedup vs baseline: 1.6525x; 1.6525x over previous
"""Trainium2 Bass kernel for nn_CLLayer (SimCLR-style contrastive loss).

Math (reference, tau=0.5):
    h1 = elu(z1 @ W1.T + b1) @ W2.T + b2 ; h2 likewise
    n1, n2 = row-normalized h1, h2
    l1_i = log(sum_j exp(2*n1_i.n1_j) + sum_j exp(2*n1_i.n2_j) - e^2) - 2*n1_i.n2_i
    l2_i = log(sum_j exp(2*n2_i.n2_j) + colsum_i exp(2*S12) - e^2) - 2*n1_i.n2_i
    out = 0.5*(l1+l2)

Sharding: row-parallel over N=8192 (1024 rows/core, 8 cores).
Each core: projects its row block (bf16 matmuls), normalizes, scales by 64 and
casts to fp8e4, AllGathers the fp8 embeddings, computes its row-strip of the
three distinct similarity products (S11, S12, S22) with fp8 DoubleRow matmuls
(2 k-tiles per MM, 2x PE throughput), exp(2/4096 * dot)+row-sums on the fly,
column-sums of exp(2*S12) via a ReduceScatter (between2 = between.T so l2's
"between" row sums are column sums of S12's exp).  Only 3 of 4 N^2*D products
are needed.

Host-side prep: transposes z blocks / weights to K-major (PE wants K on
partitions), casts projection operands to bf16, and folds the ELU "-1" into an
adjusted fc2 bias (b2' = b2 - fc2_w.sum(1)) so ELU is computed as
relu(x) + exp(min(x,0)) without the subtract (device ELU' = elu + 1).
"""

import math
import os
from functools import lru_cache

import ml_dtypes
import numpy as np

import concourse.bacc as bacc
import concourse.bass as bass
import concourse.mybir as mybir
import concourse.tile as tile
from concourse.bass_utils import run_bass_kernel_spmd

N, D = 8192, 1024
NCORES = 8
BLK = N // NCORES  # 1024
P = 128
KO = D // P  # 8 k-tiles
NT = BLK // P  # 8 i-tiles per core
JC = N // 512  # 16 j-chunks of 512
E2 = float(np.exp(2.0))  # exp(1/tau), tau=0.5
SC = 64.0  # fp8 embedding scale; dots come out scaled by SC*SC
E2S = 2.0 / (SC * SC)  # exp() scale undoing the fp8 scaling
BF = mybir.dt.bfloat16
F8 = mybir.dt.float8e4
F32 = mybir.dt.float32
AF = mybir.ActivationFunctionType
ALU = mybir.AluOpType
DR = mybir.MatmulPerfMode.DoubleRow


def _build():
    nc = bacc.Bacc("TRN2", target_bir_lowering=False, debug=False, num_devices=NCORES)

    z1t = nc.dram_tensor("z1t", [D, BLK], BF, kind="ExternalInput")
    z2t = nc.dram_tensor("z2t", [D, BLK], BF, kind="ExternalInput")
    w1t = nc.dram_tensor("w1t", [D, D], BF, kind="ExternalInput")
    w2t = nc.dram_tensor("w2t", [D, D], BF, kind="ExternalInput")
    b1 = nc.dram_tensor("b1", [D], F32, kind="ExternalInput")
    b2p = nc.dram_tensor("b2p", [D], F32, kind="ExternalInput")
    out = nc.dram_tensor("out", [BLK], F32, kind="ExternalOutput")

    kp = lambda ap: ap.rearrange("(ko ki) x -> ki ko x", ki=P)  # K-major -> [128, KO, x]
    pt = lambda ap: ap.rearrange("(t p) -> p t", p=P)  # [1024] -> [128, 8]
    JP = JC // 2  # 8 j-chunk-pairs of 1024

    with tile.TileContext(nc) as tc:
        with (
            tc.tile_pool(name="consts", bufs=1) as consts,
            tc.tile_pool(name="mats", bufs=1) as mats,
            tc.tile_pool(name="strip", bufs=1) as strip,
            tc.tile_pool(name="scratch", bufs=2) as scratch,
            tc.tile_pool(name="rhs", bufs=3) as rhsp,
            tc.tile_pool(name="expp", bufs=2) as expp,
            tc.tile_pool(name="small", bufs=1) as small,
            tc.tile_pool(name="psA", bufs=3, space="PSUM") as psA,
            tc.tile_pool(name="psB", bufs=2, space="PSUM") as psB,
            tc.tile_pool(name="dram", bufs=1, space="DRAM") as dram,
        ):
            # ---------------- constants ----------------
            w1_sb = consts.tile([P, KO, D], BF)
            w2_sb = consts.tile([P, KO, D], BF)
            nc.sync.dma_start(w1_sb[:], kp(w1t[:]))
            nc.sync.dma_start(w2_sb[:], kp(w2t[:]))
            b1_sb = consts.tile([P, KO], F32)
            b2_sb = consts.tile([P, KO], F32)
            nc.sync.dma_start(b1_sb[:], pt(b1[:]))
            nc.sync.dma_start(b2_sb[:], pt(b2p[:]))
            ones_bf = consts.tile([P, 1], BF)
            ones_f = consts.tile([P, 1], F32)
            nc.vector.memset(ones_bf[:], 1.0)
            nc.vector.memset(ones_f[:], 1.0)

            h_sb = mats.tile([P, KO, BLK], BF, tag="h")  # layer-2 out, pre-normalize
            ln1 = mats.tile([P, KO, BLK], F8, tag="ln1")  # 64 * n1, fp8
            ln2 = mats.tile([P, KO, BLK], F8, tag="ln2")  # 64 * n2, fp8

            ag1_in = dram.tile([D, BLK], F8)
            ag2_in = dram.tile([D, BLK], F8)
            ag1_out = dram.tile([NCORES, D, BLK], F8, addr_space="Shared")
            ag2_out = dram.tile([NCORES, D, BLK], F8, addr_space="Shared")
            rs_in = dram.tile([N], F32)
            rs_out = dram.tile([BLK], F32)
            rn_dram = dram.tile([2, BLK], BF)
            p_dram = dram.tile([BLK], F32)

            # ------------ projection + normalize (into ln fp8), per tensor ------------
            def project(z_at, elu_sb, ln_sb, rn_slot):
                # layer 1: a1T[o, i] = W1T.T @ zT (K=d);
                # elu+1 = relu(a+b1) + min(exp(a+b1), 1)
                for ot in range(KO):
                    ps = psA.tile([P, 1024], F32, tag="ps_big")
                    for ch in range(2):
                        sl = bass.ts(ch, 512)
                        for kt in range(KO):
                            nc.tensor.matmul(
                                ps[:, sl],
                                w1_sb[:, kt, bass.ts(ot, P)],
                                z_at(kt, ch),
                                start=(kt == 0),
                                stop=(kt == KO - 1),
                            )
                    bcol = b1_sb[:, ot : ot + 1]
                    e_t = scratch.tile([P, 1024], F32, tag="e_t")
                    r_t = scratch.tile([P, 1024], F32, tag="r_t")
                    nc.scalar.activation(e_t[:], ps[:], AF.Exp, bias=bcol)
                    nc.scalar.activation(r_t[:], ps[:], AF.Relu, bias=bcol)
                    nc.vector.tensor_scalar(e_t[:], e_t[:], 1.0, None, ALU.min)
                    nc.vector.tensor_tensor(elu_sb[:, ot, :], e_t[:], r_t[:], ALU.add)
                # layer 2 -> h_sb (pre-normalization)
                for ot in range(KO):
                    ps = psA.tile([P, 1024], F32, tag="ps_big")
                    for ch in range(2):
                        sl = bass.ts(ch, 512)
                        for kt in range(KO):
                            nc.tensor.matmul(
                                ps[:, sl],
                                w2_sb[:, kt, bass.ts(ot, P)],
                                elu_sb[:, kt, bass.ds(ch * 512, 512)],
                                start=(kt == 0),
                                stop=(kt == KO - 1),
                            )
                    nc.vector.tensor_scalar(
                        h_sb[:, ot, :], ps[:], b2_sb[:, ot : ot + 1], None, ALU.add
                    )
                # sumsq over d (partitions) via ones-matmul on Square(h)
                ssps = [psB.tile([1, 512], F32, name=f"ssps{_c}", tag="ps_small") for _c in range(2)]
                for kt in range(KO):
                    sq = scratch.tile([P, BLK], BF, tag="sq")
                    nc.scalar.activation(sq[:], h_sb[:, kt, :], AF.Square)
                    for ch in range(2):
                        nc.tensor.matmul(
                            ssps[ch][:],
                            ones_bf[:],
                            sq[:, bass.ts(ch, 512)],
                            start=(kt == 0),
                            stop=(kt == KO - 1),
                        )
                # rn = 64/||h|| per column, one Newton step on top of 1/sqrt
                rn_bf = small.tile([1, BLK], BF, tag="rn_bf")
                for ch in range(2):
                    sl = bass.ts(ch, 512)
                    ssq_c = small.tile([1, 512], F32, tag="ssq_c", name=f"ssq_c{ch}")
                    nrm_c = small.tile([1, 512], F32, tag="nrm_c", name=f"nrm_c{ch}")
                    y_c = small.tile([1, 512], F32, tag="y_c", name=f"y_c{ch}")
                    t1_c = small.tile([1, 512], F32, tag="t1_c", name=f"t1_c{ch}")
                    nc.vector.tensor_copy(ssq_c[:], ssps[ch][:])
                    nc.scalar.activation(nrm_c[:], ssps[ch][:], AF.Sqrt)
                    nc.vector.reciprocal(y_c[:], nrm_c[:])
                    nc.vector.tensor_tensor(t1_c[:], y_c[:], y_c[:], ALU.mult)
                    nc.vector.tensor_tensor(t1_c[:], t1_c[:], ssq_c[:], ALU.mult)
                    nc.vector.tensor_scalar(t1_c[:], t1_c[:], -0.5, 1.5, ALU.mult, ALU.add)
                    nc.vector.tensor_tensor(t1_c[:], y_c[:], t1_c[:], ALU.mult)
                    nc.vector.tensor_scalar(t1_c[:], t1_c[:], SC, None, ALU.mult)
                    nc.vector.tensor_copy(rn_bf[:, sl], t1_c[:])
                nc.scalar.dma_start(rn_dram[rn_slot : rn_slot + 1, :], rn_bf[:])
                rn_bc = scratch.tile([P, BLK], BF, tag="rnbc", bufs=1)
                nc.scalar.dma_start(rn_bc[:], rn_dram[rn_slot : rn_slot + 1, :].to_broadcast((P, BLK)))
                for kt in range(KO):
                    nc.vector.tensor_tensor(ln_sb[:, kt, :], h_sb[:, kt, :], rn_bc[:], ALU.mult)

            rg = [list(range(NCORES))]
            # z1 into its slot; z2 into the (idle until pass A) rhs-pool slots so
            # both projections can interleave on the PE.
            z_sb = mats.tile([P, KO, BLK], BF, tag="zt")
            nc.sync.dma_start(z_sb[:], kp(z1t[:]))
            z2a = rhsp.tile([P, KO, 512], BF, tag="rhsz", name="z2a")
            z2b = rhsp.tile([P, KO, 512], BF, tag="rhsz", name="z2b")
            nc.sync.dma_start(z2a[:], kp(z2t[:, 0:512]))
            nc.sync.dma_start(z2b[:], kp(z2t[:, 512:1024]))
            elu1 = mats.tile([P, KO, BLK], BF, tag="elu")
            project(lambda kt, ch: z_sb[:, kt, bass.ds(ch * 512, 512)], elu1, ln1, 0)
            nc.scalar.dma_start(kp(ag1_in[:]), ln1[:])
            nc.gpsimd.collective_compute(
                "AllGather", ALU.bypass, replica_groups=rg,
                ins=[ag1_in[:].opt()], outs=[ag1_out[:].opt()],
            )
            # elu2 reuses the z1 slot (z1 dead after its layer 1)
            elu2 = mats.tile([P, KO, BLK], BF, tag="zt", name="elu2")
            project(lambda kt, ch: (z2a if ch == 0 else z2b)[:, kt, :], elu2, ln2, 1)
            nc.scalar.dma_start(kp(ag2_in[:]), ln2[:])
            nc.gpsimd.collective_compute(
                "AllGather", ALU.bypass, replica_groups=rg,
                ins=[ag2_in[:].opt()], outs=[ag2_out[:].opt()],
            )

            # ---------------- p_i = ln1_i . ln2_i (local diag of S12, x4096) --------
            pps = [psB.tile([1, 512], F32, name=f"pps{_c}", tag="ps_small") for _c in range(2)]
            for kt in range(KO):
                q = scratch.tile([P, BLK], BF, tag="sq")
                nc.vector.tensor_tensor(q[:], ln1[:, kt, :], ln2[:, kt, :], ALU.mult)
                for ch in range(2):
                    nc.tensor.matmul(
                        pps[ch][:],
                        ones_bf[:],
                        q[:, bass.ts(ch, 512)],
                        start=(kt == 0),
                        stop=(kt == KO - 1),
                    )
            for ch in range(2):
                p_c = small.tile([1, 512], F32, tag="ssq_c", name=f"p_c{ch}")
                nc.vector.tensor_copy(p_c[:], pps[ch][:])
                nc.gpsimd.dma_start(p_dram[ch * 512 : (ch + 1) * 512], p_c[:])

            # rowsum partials, one column per j-chunk-pair
            r11p = strip.tile([P, NT, JP], F32)
            r12p = strip.tile([P, NT, JP], F32)
            r22p = strip.tile([P, NT, JP], F32)

            def rhs_pair(ag, jp):
                a = rhsp.tile([P, KO, 512], F8, tag="rhs", name=f"rhs_a{jp}")
                b = rhsp.tile([P, KO, 512], F8, tag="rhs", name=f"rhs_b{jp}")
                blk = kp(ag[jp])
                nc.sync.dma_start(a[:], blk[:, :, 0:512])
                nc.sync.dma_start(b[:], blk[:, :, 512:1024])
                return a, b

            def sim_iter(lhs, tt, rta, rtb, accum, csj=None):
                ps = psA.tile([P, 1024], F32, tag="ps_big", name="ps_sim")
                for ch, rt in ((0, rta), (1, rtb)):
                    sl = bass.ts(ch, 512)
                    for kt in range(0, KO, 2):
                        nc.tensor.matmul(
                            ps[:, sl],
                            lhs[:, kt : kt + 2, bass.ts(tt, P)],
                            rt[:, kt : kt + 2, :],
                            start=(kt == 0),
                            stop=(kt == KO - 2),
                            perf_mode=DR,
                        )
                ex = expp.tile([P, 1024], F32, tag="ex")
                nc.scalar.activation(ex[:], ps[:], AF.Exp, scale=E2S, accum_out=accum)
                if csj is not None:
                    nc.vector.tensor_tensor(csj[:], csj[:], ex[:], ALU.add)

            # ---- pass A: S11 (lhs ln1, rhs gathered ln1) ----
            for jp in range(JP):
                rta, rtb = rhs_pair(ag1_out, jp)
                for tt in range(NT):
                    sim_iter(ln1, tt, rta, rtb, r11p[:, tt, jp : jp + 1])

            # ---- pass B1: S12 (lhs ln1, rhs gathered ln2) + incremental colsums ----
            for jp in range(JP):
                rta, rtb = rhs_pair(ag2_out, jp)
                csj = expp.tile([P, 1024], F32, tag="cs", name=f"cs{jp}", bufs=2)
                nc.vector.memset(csj[:], 0.0)
                for tt in range(NT):
                    sim_iter(ln1, tt, rta, rtb, r12p[:, tt, jp : jp + 1], csj=csj)
                # this 1024-wide slice of colsums is complete -> reduce over partitions
                for h in range(2):
                    cp = psB.tile([1, 512], F32, tag="ps_small", name=f"cp{jp}_{h}")
                    nc.tensor.matmul(
                        cp[:], ones_f[:], csj[:, bass.ds(h * 512, 512)],
                        start=True, stop=True,
                    )
                    cst = scratch.tile([1, 512], F32, tag="cst", bufs=2, name=f"cst{jp}_{h}")
                    nc.vector.tensor_copy(cst[:], cp[:])
                    nc.gpsimd.dma_start(
                        rs_in[(jp * 2 + h) * 512 : (jp * 2 + h + 1) * 512], cst[:]
                    )
            nc.gpsimd.collective_compute(
                "ReduceScatter", ALU.add, replica_groups=rg,
                ins=[rs_in[:].opt()], outs=[rs_out[:].opt()],
            )

            # ---- pass B2: S22 (lhs ln2, rhs gathered ln2); RS overlaps this ----
            for jp in range(JP):
                rta, rtb = rhs_pair(ag2_out, jp)
                for tt in range(NT):
                    sim_iter(ln2, tt, rta, rtb, r22p[:, tt, jp : jp + 1])

            # ---------------- final loss ----------------
            r11 = small.tile([P, NT], F32, tag="r11")
            r12 = small.tile([P, NT], F32, tag="r12")
            r22 = small.tile([P, NT], F32, tag="r22")
            nc.vector.reduce_sum(r11[:], r11p[:], axis=mybir.AxisListType.X)
            nc.vector.reduce_sum(r12[:], r12p[:], axis=mybir.AxisListType.X)
            nc.vector.reduce_sum(r22[:], r22p[:], axis=mybir.AxisListType.X)
            c12 = small.tile([P, NT], F32, tag="c12")
            nc.sync.dma_start(c12[:], pt(rs_out[:]))
            p2 = small.tile([P, NT], F32, tag="p2")
            nc.sync.dma_start(p2[:], pt(p_dram[:]))

            d1 = small.tile([P, NT], F32, tag="d1")
            d2 = small.tile([P, NT], F32, tag="d2")
            nc.vector.tensor_tensor(d1[:], r11[:], r12[:], ALU.add)
            nc.vector.tensor_scalar(d1[:], d1[:], -E2, None, ALU.add)
            nc.vector.tensor_tensor(d2[:], r22[:], c12[:], ALU.add)
            nc.vector.tensor_scalar(d2[:], d2[:], -E2, None, ALU.add)
            l1 = small.tile([P, NT], F32, tag="l1")
            l2 = small.tile([P, NT], F32, tag="l2")
            nc.scalar.activation(l1[:], d1[:], AF.Ln)
            nc.scalar.activation(l2[:], d2[:], AF.Ln)
            loss = small.tile([P, NT], F32, tag="loss")
            nc.vector.tensor_tensor(loss[:], l1[:], l2[:], ALU.add)
            nc.vector.tensor_scalar(loss[:], loss[:], 0.5, None, ALU.mult)
            pm = small.tile([P, NT], F32, tag="pm")
            nc.vector.tensor_scalar(pm[:], p2[:], -E2S, None, ALU.mult)
            nc.vector.tensor_tensor(loss[:], loss[:], pm[:], ALU.add)
            nc.sync.dma_start(pt(out[:]), loss[:])

    nc.finalize()
    return nc


@lru_cache(maxsize=1)
def _built():
    return _build()


def _prep_inputs(z1, z2, fc1_w, fc1_b, fc2_w, fc2_b):
    bf = ml_dtypes.bfloat16
    w1t = np.ascontiguousarray(np.asarray(fc1_w, np.float32).T).astype(bf)
    w2t = np.ascontiguousarray(np.asarray(fc2_w, np.float32).T).astype(bf)
    b1 = np.asarray(fc1_b, np.float32)
    b2p = (np.asarray(fc2_b, np.float32) - np.asarray(fc2_w, np.float32).sum(axis=1)).astype(
        np.float32
    )
    in_maps = []
    for c in range(NCORES):
        sl = slice(c * BLK, (c + 1) * BLK)
        in_maps.append(
            {
                "z1t": np.ascontiguousarray(np.asarray(z1[sl], np.float32).T).astype(bf),
                "z2t": np.ascontiguousarray(np.asarray(z2[sl], np.float32).T).astype(bf),
                "w1t": w1t,
                "w2t": w2t,
                "b1": b1,
                "b2p": b2p,
            }
        )
    return in_maps


def _install_ntff_shim():
    """Register the axon NTFF profile hook (antenv.axon_hooks is absent in
    this image; rebuild it from trn_agent_boot's ctypes recipe)."""
    import sys
    import types

    if "antenv.axon_hooks" in sys.modules:
        return True
    try:
        import antenv
        from trn_agent_boot.trn_boot import _ntff_profile_via_ctypes

        hook = _ntff_profile_via_ctypes("/opt/axon/libaxon_pjrt.so")
        if hook is None:
            return False
        m = types.ModuleType("antenv.axon_hooks")
        m._hook = hook
        m.get_axon_ntff_profile_hook = lambda: m._hook
        m.set_axon_ntff_profile_hook = lambda h: setattr(m, "_hook", h)
        sys.modules["antenv.axon_hooks"] = m
        antenv.axon_hooks = m
        # artifact upload needs egress; neuter it for local profiling
        import concourse.bass_utils as _bu

        _bu.upload_artifacts = lambda tmpdir: f"file://{tmpdir}"
        return True
    except Exception as e:
        print(f"ntff shim unavailable: {e!r}")
        return False


def _run(in_maps, trace=False):
    nc = _built()
    if trace and not _install_ntff_shim():
        trace = False
    last = None
    for attempt in range(3):
        try:
            res = run_bass_kernel_spmd(nc, in_maps, list(range(NCORES)), trace=trace)
            if all(np.isfinite(res.results[c]["out"]).all() for c in range(NCORES)):
                return res
            print("nonfinite output, retrying")
        except Exception as e:  # device occasionally wedged from a prior process
            last = e
            if "UNRECOVERABLE" not in str(e) and "UNAVAILABLE" not in str(e):
                raise
            print(f"device error (attempt {attempt}): retrying")
    if last is not None:
        raise last
    return res


def kernel(z1, z2, fc1_w, fc1_b, fc2_w, fc2_b):
    in_maps = _prep_inputs(z1, z2, fc1_w, fc1_b, fc2_w, fc2_b)
    res = _run(in_maps, trace=os.environ.get("KERNEL_TRACE", "") == "1")
    if res.exec_time_ns is not None:
        print(f"HW exec time: {res.exec_time_ns} ns")
    out = np.concatenate([res.results[c]["out"] for c in range(NCORES)])
    return out.astype(np.float32)
